# revision 1
# baseline (speedup 1.0000x reference)
"""Sliding-window GQA attention (RoPE + attention sinks) on 8 TRN2 NeuronCores.

Problem: B=1, S=2048, H=32 q-heads, KV=8 kv-heads (GQA group 4), D=128,
sliding window 1024, causal, per-head sink logit in the softmax denominator.

Sharding: tensor-parallel over heads. Core c gets q-heads [4c, 4c+4) and kv
head c — GQA groups align exactly with cores, so there is no cross-core
communication at all. Each core computes 4 attention heads independently;
the host concatenates the 8 per-core outputs along the head axis.

Per-core kernel (all compute in bf16 with f32 PSUM accumulation):
  1. RoPE applied on device (DVE + GpSimd) in natural [s, d] layout.
  2. DMA-xbar transpose q/k to [d, s] layout for the matmuls.
  3. Key-block-outer QK^T: scoresT[k, q] in PSUM (kT block stationary,
     amortized over up to 9 query blocks).
  4. ScalarE exp(SCALE * scoresT) -> transposed probabilities pT (bf16).
  5. Sliding-window/causal masking applied post-exp as a 0/1 multiply on the
     two diagonal (partial) blocks of each key block.
  6. PV: out[q, d] = sum_j pT_j.T @ [v_j | 1]  — the pT chunk is the
     stationary operand (M=q=128) and v is extended with a ones column
     (N=129 <= 512 moving limit), so column 128 accumulates the softmax
     denominator for free.
  7. Normalize: denom += exp(sink); out *= 1/denom (per-partition scalar).
"""

import sys

sys.path.insert(0, "/opt/trn_rl_repo")

import numpy as np
import ml_dtypes

import concourse.bass as bass
from concourse import mybir, bacc
from concourse.tile import TileContext
from concourse.bass_utils import run_bass_kernel_spmd

# ---- problem constants (hardcoded per spec) ----
B, S, H, KV, D = 1, 2048, 32, 8, 128
NCORES = 8
HPC = H // NCORES          # 4 q heads per core
WINDOW = 1024
NB = S // 128              # 16 seq blocks
WB = WINDOW // 128         # 8 window blocks
SCALE = 0.08838834764831845
ROPE_BASE = 10000.0

BF16 = mybir.dt.bfloat16
F32 = mybir.dt.float32
npbf16 = ml_dtypes.bfloat16

_CACHE = {}


def _emit_body(nc, tc, pools, tensors):
    """Emit one full forward pass (4 heads) into the TileContext."""
    constp, qio, ropep, qtp, ptp, psc, pso, ostagep, smallp = pools
    q_d, k_d, v_d, cos_d, sin_d, se_d, mask_d, out_d = tensors

    # ---- shared constants ----
    maskc = constp.tile([128, 2, 128], BF16)
    nc.sync.dma_start(out=maskc, in_=mask_d.ap())
    cos_sb = constp.tile([128, NB, D], BF16)
    nc.sync.dma_start(out=cos_sb, in_=cos_d.ap().rearrange("(j p) d -> p j d", p=128))
    sin_sb = constp.tile([128, NB, D], BF16)
    nc.sync.dma_start(out=sin_sb, in_=sin_d.ap().rearrange("(j p) d -> p j d", p=128))
    se_sb = constp.tile([128, HPC], F32)
    nc.gpsimd.dma_start(
        out=se_sb, in_=bass.AP(tensor=se_d, offset=0, ap=[[0, 128], [1, HPC]])
    )
    v_sb = constp.tile([128, NB, D + 1], BF16)
    nc.sync.dma_start(out=v_sb, in_=v_d.ap().rearrange("(j p) d -> p j d", p=128))

    def rope_and_transpose(nat, xt):
        """nat: [128, NB, D] bf16 natural layout -> xt: [128, NB, D] = [d, s]."""
        swap = bass.AP(
            tensor=nat.tensor,
            offset=nat.offset + 64,
            ap=[nat.ap[0], [D, NB], [-64, 2], [1, 64]],
        )
        t1 = ropep.tile([128, NB, D], BF16, tag="t1")
        nc.gpsimd.tensor_mul(t1, swap, sin_sb)
        xr = ropep.tile([128, NB, D], BF16, tag="xr")
        nc.vector.tensor_mul(xr, nat, cos_sb)
        nc.vector.tensor_add(xr, xr, t1)
        nc.sync.dma_start_transpose(out=xt, in_=xr)

    # ---- k prep (shared by all 4 q heads) ----
    k_nat = qio.tile([128, NB, D], BF16, tag="knat", bufs=1)
    nc.sync.dma_start(out=k_nat, in_=k_d.ap().rearrange("(j p) d -> p j d", p=128))
    kT = constp.tile([128, NB, D], BF16)
    rope_and_transpose(k_nat, kT)

    for h in range(HPC):
        q_nat = qio.tile([128, NB, D], BF16, tag="qnat")
        nc.sync.dma_start(
            out=q_nat, in_=q_d.ap()[:, h, :].rearrange("(j p) d -> p j d", p=128)
        )
        qT = qtp.tile([128, NB, D], BF16)
        rope_and_transpose(q_nat, qT)

        # ---- QK^T + exp + mask, key-block-outer ----
        pts = []
        for j in range(NB):
            nq = min(j + WB, NB - 1) - j + 1  # query blocks j .. j+nq-1
            sc = psc.tile([128, WB + 1, 128], F32, tag="sc")
            sc_flat = sc[:, :nq, :].opt()
            rhs_full = qT[:, j : j + nq, :].opt()
            for c0 in range(0, nq * 128, 512):
                n = min(512, nq * 128 - c0)
                nc.tensor.matmul(
                    sc_flat[:, c0 : c0 + n],
                    kT[:, j, :],
                    rhs_full[:, c0 : c0 + n],
                    start=True,
                    stop=True,
                )
            pt = ptp.tile([128, WB + 1, 128], BF16, tag="pt")
            nc.scalar.activation(
                pt[:, :nq, :], sc[:, :nq, :], mybir.ActivationFunctionType.Exp,
                scale=SCALE,
            )
            if j + WB <= NB - 1:
                # both diagonal chunks live: causal diag (chunk 0, qblock j)
                # and window-left diag (chunk WB, qblock j+WB)
                two = bass.AP(
                    tensor=pt.tensor,
                    offset=pt.offset,
                    ap=[pt.ap[0], [WB * 128, 2], [1, 128]],
                )
                nc.vector.tensor_mul(two, two, maskc)
            else:
                nc.vector.tensor_mul(pt[:, 0, :], pt[:, 0, :], maskc[:, 0, :])
            pts.append(pt)

        # ---- PV + denominator, query-block-outer ----
        ostage = ostagep.tile([128, NB, D + 1], F32)
        for i in range(NB):
            j0 = max(0, i - WB)
            acc = pso.tile([128, D + 1], F32, tag="acc")
            for j in range(j0, i + 1):
                nc.tensor.matmul(
                    acc,
                    pts[j][:, i - j, :],
                    v_sb[:, j, :],
                    start=(j == j0),
                    stop=(j == i),
                )
            nc.vector.tensor_copy(ostage[:, i, :], acc)

        # ---- normalize: denom += exp(sink); out *= 1/denom ----
        dview = ostage[:, :, D]  # [128, NB] strided view of denominators
        dt = smallp.tile([128, NB], F32, tag="dt")
        nc.vector.tensor_scalar_add(dt, dview, se_sb[:, h : h + 1])
        rt = smallp.tile([128, NB], F32, tag="rt")
        nc.vector.reciprocal(rt, dt)
        for i in range(NB):
            nc.vector.tensor_scalar_mul(
                ostage[:, i, :D], ostage[:, i, :D], rt[:, i : i + 1]
            )
        nc.sync.dma_start(
            out=out_d.ap()[:, h, :].rearrange("(j p) d -> p j d", p=128),
            in_=ostage[:, :, :D],
        )


def build_nc(loop_r=None):
    """Build the per-core Bass graph. loop_r: if set, wrap the body in a
    For_i loop with that many serialized repetitions (for timing)."""
    nc = bacc.Bacc("TRN2", target_bir_lowering=False, num_devices=NCORES)
    q_d = nc.dram_tensor("q", [S, HPC, D], BF16, kind="ExternalInput")
    k_d = nc.dram_tensor("k", [S, D], BF16, kind="ExternalInput")
    v_d = nc.dram_tensor("vx", [S, D + 1], BF16, kind="ExternalInput")
    cos_d = nc.dram_tensor("cose", [S, D], BF16, kind="ExternalInput")
    sin_d = nc.dram_tensor("sine", [S, D], BF16, kind="ExternalInput")
    se_d = nc.dram_tensor("sinkexp", [HPC], F32, kind="ExternalInput")
    mask_d = nc.dram_tensor("maskc", [128, 2, 128], BF16, kind="ExternalInput")
    out_d = nc.dram_tensor("out", [S, HPC, D], F32, kind="ExternalOutput")
    tensors = (q_d, k_d, v_d, cos_d, sin_d, se_d, mask_d, out_d)

    with TileContext(nc) as tc:
        with (
            tc.tile_pool(name="consts", bufs=1) as constp,
            tc.tile_pool(name="qio", bufs=2) as qio,
            tc.tile_pool(name="ropep", bufs=2) as ropep,
            tc.tile_pool(name="qtp", bufs=2) as qtp,
            tc.tile_pool(name="ptp", bufs=NB + 2) as ptp,
            tc.tile_pool(name="psc", bufs=2, space="PSUM") as psc,
            tc.tile_pool(name="pso", bufs=2, space="PSUM") as pso,
            tc.tile_pool(name="ostagep", bufs=2) as ostagep,
            tc.tile_pool(name="smallp", bufs=2) as smallp,
        ):
            pools = (constp, qio, ropep, qtp, ptp, psc, pso, ostagep, smallp)
            if loop_r is None:
                _emit_body(nc, tc, pools, tensors)
            else:
                with tc.For_i(0, loop_r, 1):
                    _emit_body(nc, tc, pools, tensors)
    nc.compile()
    return nc


def _prep_in_maps(q, k, v, positions, sinks):
    pos = np.asarray(positions)[0].astype(np.float32)  # [S]
    inv_freq = 1.0 / (ROPE_BASE ** (np.arange(0, D, 2, dtype=np.float32) / D))
    ang = pos[:, None] * inv_freq[None, :]  # [S, 64]
    cos = np.cos(ang).astype(np.float32)
    sin = np.sin(ang).astype(np.float32)
    cos_ext = np.ascontiguousarray(np.concatenate([cos, cos], 1).astype(npbf16))
    sin_sgn = np.ascontiguousarray(np.concatenate([-sin, sin], 1).astype(npbf16))

    bidx = np.arange(128)
    mr = (bidx[:, None] <= bidx[None, :]).astype(npbf16)  # causal diag: k<=q
    ml = (bidx[:, None] > bidx[None, :]).astype(npbf16)   # window-left diag: k>q
    maskc = np.ascontiguousarray(np.stack([mr, ml], axis=1))  # [128, 2, 128]

    sinkexp = np.exp(np.asarray(sinks).astype(np.float32))  # [H]

    q0 = np.asarray(q)[0].astype(npbf16)   # [S, H, D]
    k0 = np.asarray(k)[0].astype(npbf16)   # [S, KV, D]
    v0 = np.asarray(v)[0].astype(np.float32)
    ones = np.ones((S, 1), np.float32)

    in_maps = []
    for c in range(NCORES):
        vx = np.concatenate([v0[:, c, :], ones], axis=1).astype(npbf16)
        in_maps.append(
            {
                "q": np.ascontiguousarray(q0[:, HPC * c : HPC * (c + 1), :]),
                "k": np.ascontiguousarray(k0[:, c, :]),
                "vx": np.ascontiguousarray(vx),
                "cose": cos_ext,
                "sine": sin_sgn,
                "sinkexp": np.ascontiguousarray(sinkexp[HPC * c : HPC * (c + 1)]),
                "maskc": maskc,
            }
        )
    return in_maps


def kernel(q, k, v, positions, sinks):
    if "nc" not in _CACHE:
        _CACHE["nc"] = build_nc()
    nc = _CACHE["nc"]
    in_maps = _prep_in_maps(q, k, v, positions, sinks)
    res = run_bass_kernel_spmd(nc, in_maps, core_ids=list(range(NCORES)))
    out = np.empty((B, S, H, D), np.float32)
    for c in range(NCORES):
        out[0, :, HPC * c : HPC * (c + 1), :] = res.results[c]["out"]
    return out


# revision 25
# speedup vs baseline: 46.8505x; 46.8505x over previous
"""Sliding-window GQA attention (RoPE + attention sinks) on 8 TRN2 NeuronCores.

Problem: B=1, S=2048, H=32 q-heads, KV=8 kv-heads (GQA group 4), D=128,
sliding window 1024, causal, per-head sink logit in the softmax denominator.

Sharding: tensor-parallel over heads. Core c gets q-heads [4c, 4c+4) and kv
head c — GQA groups align exactly with cores, so there is no cross-core
communication at all. Each core computes 4 attention heads independently;
the host concatenates the 8 per-core outputs along the head axis.

Per-core kernel (all compute in bf16 with f32 PSUM accumulation):
  1. RoPE applied on device (DVE + GpSimd) in natural [s, d] layout.
  2. DMA-xbar transpose q/k to [d, s] layout for the matmuls.
  3. Key-block-outer QK^T: scoresT[k, q] in PSUM (kT block stationary,
     amortized over up to 9 query blocks).
  4. ScalarE exp(SCALE * scoresT) -> transposed probabilities pT (bf16).
  5. Sliding-window/causal masking applied post-exp as a 0/1 multiply on the
     two diagonal (partial) blocks of each key block (DVE/GpSimd alternating).
  6. PV: out[q, d] = sum_j pT_j.T @ [v_j | 1]  — the pT chunk is the
     stationary operand (M=q=128) and v is extended with a ones column
     (N=129 <= 512 moving limit), so column 128 accumulates the softmax
     denominator for free.
  7. Normalize: denom += exp(sink); out *= 1/denom (per-partition scalar).

Heads are processed in pairs with their block loops interleaved so the
per-block PE->ACT->mask chain of one head overlaps the other head's.
"""

import sys

sys.path.insert(0, "/opt/trn_rl_repo")

import numpy as np
import ml_dtypes

import concourse.bass as bass
from concourse import mybir, bacc
from concourse.tile import TileContext
from concourse.bass_utils import run_bass_kernel_spmd

# ---- problem constants (hardcoded per spec) ----
B, S, H, KV, D = 1, 2048, 32, 8, 128
NCORES = 8
HPC = H // NCORES          # 4 q heads per core
WINDOW = 1024
NB = S // 128              # 16 seq blocks
WB = WINDOW // 128         # 8 window blocks
SCALE = 0.08838834764831845
ROPE_BASE = 10000.0

BF16 = mybir.dt.bfloat16
F32 = mybir.dt.float32
npbf16 = ml_dtypes.bfloat16

_CACHE = {}
SPLIT_PREP = True
SPLIT_NORM = True
NORM_CHUNK = 8
PAIRED = True
GROUPW = 2  # heads interleaved per group
BODY_REPS = 1
PV_LAG = 1
MASK_ENGINE = "split"  # "split" | "alt" | "dve" | "gpsimd"
INTERLEAVE = True     # interleave head pairs in block loops
ROPE_T1_ENGINE = "gpsimd"  # "gpsimd" | "dve"


def _emit_body(nc, tc, pools, tensors):
    """Emit one full forward pass (4 heads) into the TileContext."""
    constp, qio, ropep, qtp, ptp, psc, pso, ostagep, smallp = pools
    q_d, k_d, v_d, cos_d, sin_d, se_d, mask_d, out_d = tensors

    # ---- shared constants (rope-critical tensors first; k loaded before all) ----
    k_nat = qio.tile([128, NB, D], BF16, tag="knat", bufs=1)
    nc.sync.dma_start(out=k_nat, in_=k_d.ap().rearrange("(j p) d -> p j d", p=128))
    cos_sb = constp.tile([128, NB, D], BF16)
    nc.sync.dma_start(out=cos_sb, in_=cos_d.ap().rearrange("(j p) d -> p j d", p=128))
    sin_sb = constp.tile([128, NB, D], BF16)
    nc.sync.dma_start(out=sin_sb, in_=sin_d.ap().rearrange("(j p) d -> p j d", p=128))
    def rope_and_transpose(nat, xt, tagsuf, t1_engine=None):
        """nat: [128, NB, D] bf16 natural layout -> xt: [128, NB, D] = [d, s]."""
        if t1_engine is None:
            t1_engine = nc.gpsimd if ROPE_T1_ENGINE == "gpsimd" else nc.vector
        swap = bass.AP(
            tensor=nat.tensor,
            offset=nat.offset + 64,
            ap=[nat.ap[0], [D, NB], [-64, 2], [1, 64]],
        )
        t1 = ropep.tile([128, NB, D], BF16, tag="t1" + tagsuf,
                        bufs=1 if tagsuf == "k" else 2)
        xr = ropep.tile([128, NB, D], BF16, tag="xr" + tagsuf,
                        bufs=1 if tagsuf == "k" else 2)
        if not SPLIT_PREP:
            t1_engine.tensor_mul(t1, swap, sin_sb)
            nc.vector.tensor_mul(xr, nat, cos_sb)
            nc.vector.tensor_add(xr, xr, t1)
            nc.sync.dma_start_transpose(out=xt, in_=xr)
            return
        swap0 = bass.AP(tensor=nat.tensor, offset=nat.offset + 64,
                        ap=[nat.ap[0], [D, WB + 1], [-64, 2], [1, 64]])
        swap1 = bass.AP(tensor=nat.tensor,
                        offset=nat.offset + 64 + (WB + 1) * D,
                        ap=[nat.ap[0], [D, NB - WB - 1], [-64, 2], [1, 64]])
        for sw, lo, hi in ((swap0, 0, WB + 1), (swap1, WB + 1, NB)):
            t1_engine.tensor_mul(t1[:, lo:hi, :], sw, sin_sb[:, lo:hi, :])
            nc.vector.tensor_mul(xr[:, lo:hi, :], nat[:, lo:hi, :],
                                 cos_sb[:, lo:hi, :])
            nc.vector.tensor_add(xr[:, lo:hi, :], xr[:, lo:hi, :],
                                 t1[:, lo:hi, :])
            nc.sync.dma_start_transpose(out=xt[:, lo:hi, :],
                                        in_=xr[:, lo:hi, :])

    # ---- k rope (DVE t1: keeps the startup chain off the slow GpSimd) ----
    kT = constp.tile([128, NB, D], BF16)
    rope_and_transpose(k_nat, kT, "k", t1_engine=nc.vector)

    def qkt_exp_mask(h, j, qT):
        nq = min(j + WB, NB - 1) - j + 1  # query blocks j .. j+nq-1
        sc = psc.tile([128, WB + 1, 128], F32, tag="sc")
        sc_flat = sc[:, :nq, :].opt()
        rhs_full = qT[:, j : j + nq, :].opt()
        for c0 in range(0, nq * 128, 512):
            n = min(512, nq * 128 - c0)
            nc.tensor.matmul(
                sc_flat[:, c0 : c0 + n],
                kT[:, j, :],
                rhs_full[:, c0 : c0 + n],
                start=True,
                stop=True,
            )
        pt = ptp.tile([128, WB + 1, 128], BF16, tag="pt")
        nc.scalar.activation(
            pt[:, :nq, :], sc[:, :nq, :], mybir.ActivationFunctionType.Exp,
            scale=SCALE,
        )
        if MASK_ENGINE == "split":
            # causal diag feeds PV immediately -> fast DVE; window-left diag
            # is consumed WB steps later -> slack absorbs slower GpSimd
            nc.vector.tensor_mul(pt[:, 0, :], pt[:, 0, :], maskc[:, 0, :])
            if j + WB <= NB - 1:
                nc.gpsimd.tensor_mul(pt[:, WB, :], pt[:, WB, :], maskc[:, 1, :])
            return pt
        if MASK_ENGINE == "alt":
            eng = nc.vector if (j % 2 == 0) else nc.gpsimd
        else:
            eng = nc.vector if MASK_ENGINE == "dve" else nc.gpsimd
        if j + WB <= NB - 1:
            # both diagonal chunks live: causal diag (chunk 0, qblock j)
            # and window-left diag (chunk WB, qblock j+WB)
            two = bass.AP(
                tensor=pt.tensor,
                offset=pt.offset,
                ap=[pt.ap[0], [WB * 128, 2], [1, 128]],
            )
            eng.tensor_mul(two, two, maskc)
        else:
            eng.tensor_mul(pt[:, 0, :], pt[:, 0, :], maskc[:, 0, :])
        return pt

    def pv_evac(h, i, pts, ostage):
        j0 = max(0, i - WB)
        acc = pso.tile([128, D + 1], F32, tag="acc")
        for j in range(j0, i + 1):
            nc.tensor.matmul(
                acc,
                pts[j][:, i - j, :],
                v_sb[:, j, :],
                start=(j == j0),
                stop=(j == i),
            )
        nc.vector.tensor_copy(ostage[:, i, :], acc)

    # ---- per-head fused pipeline: pv(h, j) right after qkt/exp/mask(h, j) ----
    qTs, ptss, ostages = {}, {}, {}
    for h in range(HPC):
        q_nat = qio.tile([128, NB, D], BF16, tag="qnat")
        nc.sync.dma_start(
            out=q_nat, in_=q_d.ap()[h].rearrange("(j p) d -> p j d", p=128)
        )
        qT = qtp.tile([128, NB, D], BF16, tag="qT", name=f"qT{h}")
        rope_and_transpose(q_nat, qT, "q", t1_engine=nc.vector if h == 0 else None)
        qTs[h] = qT
        ptss[h] = []
        ostages[h] = ostagep.tile(
            [128, NB, D + 1], BF16, tag="ostage", name=f"ostage{h}"
        )

    # remaining constants (not needed until first mask / first PV)
    maskc = constp.tile([128, 2, 128], BF16)
    nc.sync.dma_start(out=maskc, in_=mask_d.ap())
    v_sb = constp.tile([128, NB, D + 1], BF16)
    nc.sync.dma_start(out=v_sb, in_=v_d.ap().rearrange("(j p) d -> p j d", p=128))
    se_sb = constp.tile([128, HPC], F32)
    nc.gpsimd.dma_start(
        out=se_sb, in_=bass.AP(tensor=se_d, offset=0, ap=[[0, 128], [1, HPC]])
    )

    def normalize_store(h, lo=0, hi=NB):
        ostage = ostages[h]
        nblk = hi - lo
        dview = ostage[:, lo:hi, D]  # [128, nblk] strided denominators
        dt = smallp.tile([128, NB], F32, tag="dt")
        nc.vector.tensor_scalar_add(dt[:, :nblk], dview, se_sb[:, h : h + 1])
        rt = smallp.tile([128, NB], F32, tag="rt")
        nc.vector.reciprocal(rt[:, :nblk], dt[:, :nblk])
        for i in range(lo, hi):
            nc.vector.tensor_scalar_mul(
                ostage[:, i, :D], ostage[:, i, :D], rt[:, i - lo : i - lo + 1]
            )
        nc.sync.dma_start(
            out=out_d.ap()[h].rearrange("(j p) d -> p j d", p=128)[:, lo:hi, :],
            in_=ostage[:, lo:hi, :D],
        )

    if not PAIRED:
        for h in range(HPC):
            for j in range(NB):
                ptss[h].append(qkt_exp_mask(h, j, qTs[h]))
                if j >= PV_LAG:
                    pv_evac(h, j - PV_LAG, ptss[h], ostages[h])
            for i in range(NB - PV_LAG, NB):
                pv_evac(h, i, ptss[h], ostages[h])
            normalize_store(h)
    else:
        for h0 in range(0, HPC, GROUPW):
            pair = tuple(range(h0, h0 + GROUPW))
            for j in range(NB):
                for h in pair:
                    ptss[h].append(qkt_exp_mask(h, j, qTs[h]))
                if j >= PV_LAG:
                    for h in pair:
                        pv_evac(h, j - PV_LAG, ptss[h], ostages[h])
                if SPLIT_NORM and j >= NORM_CHUNK + PV_LAG and (
                    (j - PV_LAG) % NORM_CHUNK == 0
                ):
                    for h in pair:
                        normalize_store(h, j - PV_LAG - NORM_CHUNK, j - PV_LAG)
            for i in range(NB - PV_LAG, NB):
                for h in pair:
                    pv_evac(h, i, ptss[h], ostages[h])
            for h in pair:
                if SPLIT_NORM:
                    done = NORM_CHUNK * ((NB - 1 - PV_LAG) // NORM_CHUNK)
                    normalize_store(h, done, NB)
                else:
                    normalize_store(h)

def build_nc(loop_r=None, inline_inputs=None):
    """Build the per-core Bass graph. loop_r: if set, wrap the body in a
    For_i loop with that many serialized repetitions (for timing).
    inline_inputs: optional dict name->np.ndarray baked into the NEFF as
    Const tensors (timing mode: avoids per-call input upload)."""
    nc = bacc.Bacc("TRN2", target_bir_lowering=False, num_devices=NCORES)
    if inline_inputs is None:
        q_d = nc.dram_tensor("q", [HPC, S, D], BF16, kind="ExternalInput")
        k_d = nc.dram_tensor("k", [S, D], BF16, kind="ExternalInput")
        v_d = nc.dram_tensor("vx", [S, D + 1], BF16, kind="ExternalInput")
        cos_d = nc.dram_tensor("cose", [S, D], BF16, kind="ExternalInput")
        sin_d = nc.dram_tensor("sine", [S, D], BF16, kind="ExternalInput")
        se_d = nc.dram_tensor("sinkexp", [HPC], F32, kind="ExternalInput")
        mask_d = nc.dram_tensor("maskc", [128, 2, 128], BF16, kind="ExternalInput")
    else:
        ii = inline_inputs
        q_d = nc.inline_tensor(ii["q"], "q")
        k_d = nc.inline_tensor(ii["k"], "k")
        v_d = nc.inline_tensor(ii["vx"], "vx")
        cos_d = nc.inline_tensor(ii["cose"], "cose")
        sin_d = nc.inline_tensor(ii["sine"], "sine")
        se_d = nc.inline_tensor(ii["sinkexp"], "sinkexp")
        mask_d = nc.inline_tensor(ii["maskc"], "maskc")
    out_d = nc.dram_tensor("out", [HPC, S, D], BF16, kind="ExternalOutput")
    tensors = (q_d, k_d, v_d, cos_d, sin_d, se_d, mask_d, out_d)

    with TileContext(nc) as tc:
        with (
            tc.tile_pool(name="consts", bufs=1) as constp,
            tc.tile_pool(name="qio", bufs=3) as qio,
            tc.tile_pool(name="ropep", bufs=3) as ropep,
            tc.tile_pool(name="qtp", bufs=4) as qtp,
            tc.tile_pool(name="ptp", bufs=GROUPW * (WB + 1 + PV_LAG) + 4) as ptp,
            tc.tile_pool(name="psc", bufs=2, space="PSUM") as psc,
            tc.tile_pool(name="pso", bufs=2, space="PSUM") as pso,
            tc.tile_pool(name="ostagep", bufs=2) as ostagep,
            tc.tile_pool(name="smallp", bufs=2) as smallp,
        ):
            pools = (constp, qio, ropep, qtp, ptp, psc, pso, ostagep, smallp)
            if loop_r is None:
                _emit_body(nc, tc, pools, tensors)
            else:
                with tc.For_i(0, loop_r, 1):
                    for _rep in range(BODY_REPS):
                        _emit_body(nc, tc, pools, tensors)
    nc.compile()
    return nc


def _prep_in_maps(q, k, v, positions, sinks):
    pos = np.asarray(positions)[0].astype(np.float32)  # [S]
    inv_freq = 1.0 / (ROPE_BASE ** (np.arange(0, D, 2, dtype=np.float32) / D))
    ang = pos[:, None] * inv_freq[None, :]  # [S, 64]
    cos = np.cos(ang).astype(np.float32)
    sin = np.sin(ang).astype(np.float32)
    cos_ext = np.ascontiguousarray(np.concatenate([cos, cos], 1).astype(npbf16))
    sin_sgn = np.ascontiguousarray(np.concatenate([-sin, sin], 1).astype(npbf16))

    bidx = np.arange(128)
    mr = (bidx[:, None] <= bidx[None, :]).astype(npbf16)  # causal diag: k<=q
    ml = (bidx[:, None] > bidx[None, :]).astype(npbf16)   # window-left diag: k>q
    maskc = np.ascontiguousarray(np.stack([mr, ml], axis=1))  # [128, 2, 128]

    sinkexp = np.exp(np.asarray(sinks).astype(np.float32))  # [H]

    q0 = np.asarray(q)[0].astype(npbf16)   # [S, H, D]
    k0 = np.asarray(k)[0].astype(npbf16)   # [S, KV, D]
    v0 = np.asarray(v)[0].astype(np.float32)
    ones = np.ones((S, 1), np.float32)

    in_maps = []
    for c in range(NCORES):
        vx = np.concatenate([v0[:, c, :], ones], axis=1).astype(npbf16)
        in_maps.append(
            {
                "q": np.ascontiguousarray(
                    q0[:, HPC * c : HPC * (c + 1), :].transpose(1, 0, 2)
                ),
                "k": np.ascontiguousarray(k0[:, c, :]),
                "vx": np.ascontiguousarray(vx),
                "cose": cos_ext,
                "sine": sin_sgn,
                "sinkexp": np.ascontiguousarray(sinkexp[HPC * c : HPC * (c + 1)]),
                "maskc": maskc,
            }
        )
    return in_maps


def kernel(q, k, v, positions, sinks):
    if "nc" not in _CACHE:
        _CACHE["nc"] = build_nc()
    nc = _CACHE["nc"]
    in_maps = _prep_in_maps(q, k, v, positions, sinks)
    res = run_bass_kernel_spmd(nc, in_maps, core_ids=list(range(NCORES)))
    out = np.empty((B, S, H, D), np.float32)
    for c in range(NCORES):
        out[0, :, HPC * c : HPC * (c + 1), :] = (
            res.results[c]["out"].astype(np.float32).transpose(1, 0, 2)
        )
    return out
